# revision 4
# baseline (speedup 1.0000x reference)
"""GroupedQueryAttention (floor-score variant) Bass kernel for 8 NeuronCores.

Sharding: data-parallel over batch (2) x tensor-parallel over query heads
(32 heads -> 4 groups of 8). Core c handles batch c//4, head-group c%4.
Each head-group needs 2 KV heads (group_size=4 queries share a KV head).

Per-core math (S=2048, D=2048, hd=64):
  qT = Wq_g^T @ x^T + bq_g   (f32r matmuls, PSUM fp32 accumulation)
  kT, vT likewise; V un-transposed via PE transpose.
  scoreT[sk, sq] = K Q^T      (f32r, K=64 contraction)
  F = floor(scoreT * 0.125)   (custom DVE op, exact)
  outT[hd, sq] += V^T F       (f32r, accumulated over sk tiles)
Host transposes x per batch on the way in and out^T on the way out.
"""

import numpy as np

import concourse.bass as bass
import concourse.mybir as mybir
import concourse.tile as tile
from concourse import bacc, bass_utils, dve_ops
from concourse.dve_spec import Spec, Src0, C0, C2, lower, _has_src1
from concourse.dve_uop import DveOpSpec
from concourse.masks import make_identity

F32 = mybir.dt.float32
F32R = mybir.dt.float32r
ACT = mybir.ActivationFunctionType

B, S, D = 2, 2048, 2048
NH, GS, HD = 32, 4, 64          # heads, group size, head dim
NG = NH // GS                   # 8 kv heads total
HPC = 8                         # heads per core
KVPC = 2                        # kv heads per core
DQ = HPC * HD                   # 512 q-proj cols per core
DKV = KVPC * HD                 # 128 kv-proj cols per core
NT = D // 128                   # 16 contraction chunks
NJ = 4                          # sq chunks of 512
JW = 512
NSK = S // 128                  # 16 sk tiles
MAGIC = 12582912.0              # 1.5 * 2**23

_FLOOR_OP = None


def _get_floor_op():
    """Register the exact floor(in*imm2) custom DVE op once per process."""
    global _FLOOR_OP
    if _FLOOR_OP is not None:
        return _FLOOR_OP
    name = "FLOOR8_ANT"
    if name not in dve_ops._SUB_OPCODE_FOR_NAME:
        _z = Src0 * C2
        _r = (_z + C0) - C0
        body = _r - (_z < _r)
        spec = Spec(
            body=body,
            reference=lambda in0, in1, s0, s1, imm2: np.floor(
                in0.astype(np.float32) * imm2
            ),
        )
        row = max(dve_ops._SUB_OPCODE_FOR_NAME.values()) + 1
        dve_ops._SUB_OPCODE_FOR_NAME[name] = row
        uops = lower(spec, ver="v3")
        sha = DveOpSpec(name=name, opcode=row, uops=uops, rd1_en=_has_src1(spec)).sha(
            "v3"
        )
        op = dve_ops.DveOp(name, spec, subdim=False, uops_sha={"v3": sha})
        dve_ops.OPS.append(op)
        dve_ops.CUSTOM_DVE_SPECS[name] = spec
        _FLOOR_OP = op
    else:
        _FLOOR_OP = next(o for o in dve_ops.OPS if o.name == name)
    return _FLOOR_OP


def _build():
    floor_op = _get_floor_op()
    nc = bacc.Bacc("TRN2", target_bir_lowering=False, debug=False)

    xT = nc.dram_tensor("xT", [D, S], F32, kind="ExternalInput").ap()
    wq = nc.dram_tensor("wq", [D, DQ], F32, kind="ExternalInput").ap()
    wk = nc.dram_tensor("wk", [D, DKV], F32, kind="ExternalInput").ap()
    wv = nc.dram_tensor("wv", [D, DKV], F32, kind="ExternalInput").ap()
    bq = nc.dram_tensor("bq", [DQ], F32, kind="ExternalInput").ap()
    bk = nc.dram_tensor("bk", [DKV], F32, kind="ExternalInput").ap()
    bv = nc.dram_tensor("bv", [DKV], F32, kind="ExternalInput").ap()
    outT = nc.dram_tensor("outT", [DQ, S], F32, kind="ExternalOutput").ap()

    with tile.TileContext(nc) as tc:
        with (
            tc.tile_pool(name="weights", bufs=1) as wpool,
            tc.tile_pool(name="persist", bufs=1) as ppool,
            tc.tile_pool(name="xtp", bufs=1) as xpool,
            tc.tile_pool(name="vtmp", bufs=2) as vpool,
            tc.tile_pool(name="fpool", bufs=4) as fpool,
            tc.tile_pool(name="opool", bufs=4) as opool,
            tc.tile_pool(name="proj_ps", bufs=2, space="PSUM") as proj_ps,
            tc.tile_pool(name="score_ps", bufs=2, space="PSUM") as score_ps,
            tc.tile_pool(name="out_ps", bufs=1, space="PSUM") as out_ps,
        ):
            # --- persistent weight/bias tiles ---
            wq_sb = wpool.tile([128, NT, DQ], F32R, tag="wq")
            nc.gpsimd.dma_start(wq_sb[:], wq.rearrange("(t p) m -> p t m", p=128))
            wk_sb = wpool.tile([128, NT, DKV], F32R, tag="wk")
            nc.gpsimd.dma_start(wk_sb[:], wk.rearrange("(t p) m -> p t m", p=128))
            wv_sb = wpool.tile([128, NT, DKV], F32R, tag="wv")
            nc.gpsimd.dma_start(wv_sb[:], wv.rearrange("(t p) m -> p t m", p=128))
            bq_sb = wpool.tile([128, 4], F32, tag="bq")
            nc.sync.dma_start(bq_sb[:], bq.rearrange("(m p) -> p m", p=128))
            bk_sb = wpool.tile([128, 1], F32, tag="bk")
            nc.sync.dma_start(bk_sb[:], bk.rearrange("(m p) -> p m", p=128))
            bv_sb = wpool.tile([128, 1], F32, tag="bv")
            nc.sync.dma_start(bv_sb[:], bv.rearrange("(m p) -> p m", p=128))
            ident = wpool.tile([128, 128], F32, tag="ident")
            make_identity(nc, ident[:])

            # --- persistent activation tiles ---
            qt = [ppool.tile([64, S], F32R, name=f"qt{h}", tag=f"qt{h}") for h in range(HPC)]
            kt = [ppool.tile([64, S], F32R, name=f"kt{g}", tag=f"kt{g}") for g in range(KVPC)]
            v_sb = ppool.tile([128, NSK, DKV], F32R, tag="vsb")

            # --- projections, streamed over sq chunks ---
            for j in range(NJ):
                xt = xpool.tile([128, NT, JW], F32R, tag="xt")
                nc.gpsimd.dma_start(
                    xt[:],
                    xT.rearrange("(t p) s -> p t s", p=128)[
                        :, :, j * JW : (j + 1) * JW
                    ],
                )
                # Q projection: 4 chunks of 128 d_out
                for mc in range(4):
                    ps = proj_ps.tile([128, JW], F32, tag="proj")
                    for t in range(NT):
                        nc.tensor.matmul(
                            ps[:],
                            wq_sb[:, t, mc * 128 : (mc + 1) * 128],
                            xt[:, t, :],
                            start=(t == 0),
                            stop=(t == NT - 1),
                        )
                    for half in range(2):
                        h = mc * 2 + half
                        nc.scalar.activation(
                            qt[h][:, j * JW : (j + 1) * JW],
                            ps[half * 64 : half * 64 + 64, :],
                            ACT.Identity,
                            bias=bq_sb[half * 64 : half * 64 + 64, mc : mc + 1],
                        )
                # K projection
                ps = proj_ps.tile([128, JW], F32, tag="proj")
                for t in range(NT):
                    nc.tensor.matmul(
                        ps[:], wk_sb[:, t, :], xt[:, t, :],
                        start=(t == 0), stop=(t == NT - 1),
                    )
                for g in range(KVPC):
                    nc.scalar.activation(
                        kt[g][:, j * JW : (j + 1) * JW],
                        ps[g * 64 : g * 64 + 64, :],
                        ACT.Identity,
                        bias=bk_sb[g * 64 : g * 64 + 64, :],
                    )
                # V projection -> vT tile -> PE transpose -> v_sb
                ps = proj_ps.tile([128, JW], F32, tag="proj")
                for t in range(NT):
                    nc.tensor.matmul(
                        ps[:], wv_sb[:, t, :], xt[:, t, :],
                        start=(t == 0), stop=(t == NT - 1),
                    )
                vt_tmp = vpool.tile([128, JW], F32, tag="vt")
                nc.scalar.activation(vt_tmp[:], ps[:], ACT.Identity, bias=bv_sb[:])
                for jt in range(JW // 128):
                    tp = proj_ps.tile([128, 128], F32, tag="proj")
                    nc.tensor.transpose(tp[:], vt_tmp[:, jt * 128 : (jt + 1) * 128], ident[:])
                    nc.vector.tensor_copy(v_sb[:, j * 4 + jt, :], tp[:])

            # --- attention ---
            for h in range(HPC):
                g = h // GS
                outp = [out_ps.tile([64, JW], F32, name=f"outp{h}_{j}", tag=f"out{j}") for j in range(NJ)]
                for t in range(NSK):
                    fs = []
                    for j in range(NJ):
                        sp = score_ps.tile([128, JW], F32, tag="score")
                        nc.tensor.matmul(
                            sp[:],
                            kt[g][:, t * 128 : (t + 1) * 128],
                            qt[h][:, j * JW : (j + 1) * JW],
                            start=True, stop=True,
                        )
                        f = fpool.tile([128, JW], F32R, tag="floor")
                        nc.vector._custom_dve(
                            floor_op, out=f[:], in0=sp[:], s0=MAGIC, imm2=0.125
                        )
                        fs.append(f)
                    for j in range(NJ):
                        nc.tensor.matmul(
                            outp[j][:],
                            v_sb[:, t, g * 64 : (g + 1) * 64],
                            fs[j][:],
                            start=(t == 0), stop=(t == NSK - 1),
                        )
                for j in range(NJ):
                    ob = opool.tile([64, JW], F32, tag="ob")
                    nc.scalar.activation(ob[:], outp[j][:], ACT.Identity)
                    nc.sync.dma_start(
                        outT[h * 64 : (h + 1) * 64, j * JW : (j + 1) * JW], ob[:]
                    )

    nc.compile()
    return nc


_NC = None


def _get_nc():
    global _NC
    if _NC is None:
        _NC = _build()
    return _NC


def kernel(x, attention_mask, Wq, bq, Wk, bk, Wv, bv, **_ignored):
    x = np.asarray(x, dtype=np.float32)
    Wq = np.asarray(Wq, dtype=np.float32)
    Wk = np.asarray(Wk, dtype=np.float32)
    Wv = np.asarray(Wv, dtype=np.float32)
    bq = np.asarray(bq, dtype=np.float32)
    bk = np.asarray(bk, dtype=np.float32)
    bv = np.asarray(bv, dtype=np.float32)

    nc = _get_nc()
    in_maps = []
    for c in range(8):
        b, g = c // 4, c % 4
        qs = slice(g * DQ, (g + 1) * DQ)
        ks = slice(g * DKV, (g + 1) * DKV)
        in_maps.append(
            {
                "xT": np.ascontiguousarray(x[b].T),
                "wq": np.ascontiguousarray(Wq[:, qs]),
                "wk": np.ascontiguousarray(Wk[:, ks]),
                "wv": np.ascontiguousarray(Wv[:, ks]),
                "bq": np.ascontiguousarray(bq[qs]),
                "bk": np.ascontiguousarray(bk[ks]),
                "bv": np.ascontiguousarray(bv[ks]),
            }
        )
    res = bass_utils.run_bass_kernel_spmd(nc, in_maps, core_ids=list(range(8)))
    out = np.empty((B, S, D), dtype=np.float32)
    for c in range(8):
        b, g = c // 4, c % 4
        out[b, :, g * DQ : (g + 1) * DQ] = res.results[c]["outT"].T
    return out


# revision 7
# speedup vs baseline: 1.2658x; 1.2658x over previous
"""GroupedQueryAttention (floor-score variant) Bass kernel for 8 NeuronCores.

Sharding: data-parallel over batch (2) x tensor-parallel over query heads
(32 heads -> 4 groups of 8). Core c handles batch c//4, head-group c%4.
Each head-group needs 2 KV heads (group_size=4 queries share a KV head).

Per-core math (S=2048, D=2048, hd=64):
  qT = Wq_g^T @ x^T + bq_g   (f32r matmuls, PSUM fp32 accumulation)
  kT, vT likewise; V un-transposed via PE transpose.
  scoreT[sk, sq] = K Q^T      (f32r, K=64 contraction)
  F = floor(scoreT * 0.125)   (custom DVE op, exact)
  outT[hd, sq] += V^T F       (f32r, accumulated over sk tiles)
Host transposes x per batch on the way in and out^T on the way out.
"""

import numpy as np

import concourse.bass as bass
import concourse.mybir as mybir
import concourse.tile as tile
from concourse import bacc, bass_utils, dve_ops
from concourse.dve_spec import Spec, Src0, C0, C2, lower, _has_src1
from concourse.dve_uop import DveOpSpec
from concourse.masks import make_identity

F32 = mybir.dt.float32
F32R = mybir.dt.float32r
ACT = mybir.ActivationFunctionType

B, S, D = 2, 2048, 2048
NH, GS, HD = 32, 4, 64          # heads, group size, head dim
NG = NH // GS                   # 8 kv heads total
HPC = 8                         # heads per core
KVPC = 2                        # kv heads per core
DQ = HPC * HD                   # 512 q-proj cols per core
DKV = KVPC * HD                 # 128 kv-proj cols per core
NT = D // 128                   # 16 contraction chunks
NJ = 4                          # sq chunks of 512
JW = 512
NSK = S // 128                  # 16 sk tiles
MAGIC = 12582912.0              # 1.5 * 2**23

_FLOOR_OP = None


def _get_floor_op():
    """Register the exact floor(in*imm2) custom DVE op once per process."""
    global _FLOOR_OP
    if _FLOOR_OP is not None:
        return _FLOOR_OP
    name = "FLOOR8_ANT"
    if name not in dve_ops._SUB_OPCODE_FOR_NAME:
        _z = Src0 * C2
        _r = (_z + C0) - C0
        body = _r - (_z < _r)
        spec = Spec(
            body=body,
            reference=lambda in0, in1, s0, s1, imm2: np.floor(
                in0.astype(np.float32) * imm2
            ),
        )
        row = max(dve_ops._SUB_OPCODE_FOR_NAME.values()) + 1
        dve_ops._SUB_OPCODE_FOR_NAME[name] = row
        uops = lower(spec, ver="v3")
        sha = DveOpSpec(name=name, opcode=row, uops=uops, rd1_en=_has_src1(spec)).sha(
            "v3"
        )
        op = dve_ops.DveOp(name, spec, subdim=False, uops_sha={"v3": sha})
        dve_ops.OPS.append(op)
        dve_ops.CUSTOM_DVE_SPECS[name] = spec
        _FLOOR_OP = op
    else:
        _FLOOR_OP = next(o for o in dve_ops.OPS if o.name == name)
    return _FLOOR_OP


def _build():
    floor_op = _get_floor_op()
    nc = bacc.Bacc("TRN2", target_bir_lowering=False, debug=False)

    xT = nc.dram_tensor("xT", [D, S], F32, kind="ExternalInput").ap()
    wq = nc.dram_tensor("wq", [D, DQ], F32, kind="ExternalInput").ap()
    wk = nc.dram_tensor("wk", [D, DKV], F32, kind="ExternalInput").ap()
    wv = nc.dram_tensor("wv", [D, DKV], F32, kind="ExternalInput").ap()
    bq = nc.dram_tensor("bq", [DQ], F32, kind="ExternalInput").ap()
    bk = nc.dram_tensor("bk", [DKV], F32, kind="ExternalInput").ap()
    bv = nc.dram_tensor("bv", [DKV], F32, kind="ExternalInput").ap()
    outT = nc.dram_tensor("outT", [DQ, S], F32, kind="ExternalOutput").ap()

    with tile.TileContext(nc) as tc:
        with (
            tc.tile_pool(name="weights", bufs=1) as wpool,
            tc.tile_pool(name="persist", bufs=1) as ppool,
            tc.tile_pool(name="xtp", bufs=2) as xpool,
            tc.tile_pool(name="vtmp", bufs=2) as vpool,
            tc.tile_pool(name="fpool", bufs=6) as fpool,
            tc.tile_pool(name="opool", bufs=4) as opool,
            tc.tile_pool(name="big_ps", bufs=4, space="PSUM") as big_ps,
            tc.tile_pool(name="out_ps", bufs=1, space="PSUM") as out_ps,
        ):
            # --- persistent weight/bias tiles ---
            wq_sb = wpool.tile([128, NT, DQ], F32R, tag="wq")
            nc.gpsimd.dma_start(wq_sb[:], wq.rearrange("(t p) m -> p t m", p=128))
            wk_sb = wpool.tile([128, NT, DKV], F32R, tag="wk")
            nc.gpsimd.dma_start(wk_sb[:], wk.rearrange("(t p) m -> p t m", p=128))
            wv_sb = wpool.tile([128, NT, DKV], F32R, tag="wv")
            nc.gpsimd.dma_start(wv_sb[:], wv.rearrange("(t p) m -> p t m", p=128))
            bq_sb = wpool.tile([128, 4], F32, tag="bq")
            nc.sync.dma_start(bq_sb[:], bq.rearrange("(m p) -> p m", p=128))
            bk_sb = wpool.tile([128, 1], F32, tag="bk")
            nc.sync.dma_start(bk_sb[:], bk.rearrange("(m p) -> p m", p=128))
            bv_sb = wpool.tile([128, 1], F32, tag="bv")
            nc.sync.dma_start(bv_sb[:], bv.rearrange("(m p) -> p m", p=128))
            ident = wpool.tile([128, 128], F32, tag="ident")
            make_identity(nc, ident[:])

            # --- persistent activation tiles ---
            qt = [ppool.tile([64, S], F32R, name=f"qt{h}", tag=f"qt{h}") for h in range(HPC)]
            kt = [ppool.tile([64, S], F32R, name=f"kt{g}", tag=f"kt{g}") for g in range(KVPC)]
            v_sb = ppool.tile([128, NSK, DKV], F32R, tag="vsb")

            # --- projections, streamed over sq chunks ---
            PW = 256
            for j in range(2 * NJ):
                xt = xpool.tile([128, NT, PW], F32R, tag="xt")
                nc.gpsimd.dma_start(
                    xt[:],
                    xT.rearrange("(t p) s -> p t s", p=128)[
                        :, :, j * PW : (j + 1) * PW
                    ],
                )
                # Q projection: 4 chunks of 128 d_out
                for mc in range(4):
                    ps = big_ps.tile([128, PW], F32, tag="big")
                    for t in range(NT):
                        nc.tensor.matmul(
                            ps[:],
                            wq_sb[:, t, mc * 128 : (mc + 1) * 128],
                            xt[:, t, :],
                            start=(t == 0),
                            stop=(t == NT - 1),
                        )
                    for half in range(2):
                        h = mc * 2 + half
                        nc.scalar.activation(
                            qt[h][:, j * PW : (j + 1) * PW],
                            ps[half * 64 : half * 64 + 64, :],
                            ACT.Identity,
                            bias=bq_sb[half * 64 : half * 64 + 64, mc : mc + 1],
                        )
                # K projection
                ps = big_ps.tile([128, PW], F32, tag="big")
                for t in range(NT):
                    nc.tensor.matmul(
                        ps[:], wk_sb[:, t, :], xt[:, t, :],
                        start=(t == 0), stop=(t == NT - 1),
                    )
                for g in range(KVPC):
                    nc.scalar.activation(
                        kt[g][:, j * PW : (j + 1) * PW],
                        ps[g * 64 : g * 64 + 64, :],
                        ACT.Identity,
                        bias=bk_sb[g * 64 : g * 64 + 64, :],
                    )
                # V projection -> vT tile -> PE transpose -> v_sb
                ps = big_ps.tile([128, PW], F32, tag="big")
                for t in range(NT):
                    nc.tensor.matmul(
                        ps[:], wv_sb[:, t, :], xt[:, t, :],
                        start=(t == 0), stop=(t == NT - 1),
                    )
                vt_tmp = vpool.tile([128, PW], F32, tag="vt")
                nc.scalar.activation(vt_tmp[:], ps[:], ACT.Identity, bias=bv_sb[:])
                for jt in range(PW // 128):
                    tp = big_ps.tile([128, 128], F32, tag="big")
                    nc.tensor.transpose(tp[:], vt_tmp[:, jt * 128 : (jt + 1) * 128], ident[:])
                    nc.vector.tensor_copy(v_sb[:, j * 2 + jt, :], tp[:])

            # --- attention ---
            for h in range(HPC):
                g = h // GS
                outp = [out_ps.tile([64, JW], F32, name=f"outp{h}_{j}", tag=f"out{j}") for j in range(NJ)]
                for t in range(NSK):
                    fs = []
                    for j in range(NJ):
                        sp = big_ps.tile([128, JW], F32, tag="big")
                        nc.tensor.matmul(
                            sp[:],
                            kt[g][:, t * 128 : (t + 1) * 128],
                            qt[h][:, j * JW : (j + 1) * JW],
                            start=True, stop=True,
                        )
                        f = fpool.tile([128, JW], F32R, tag="floor")
                        nc.vector._custom_dve(
                            floor_op, out=f[:], in0=sp[:], s0=MAGIC, imm2=0.125
                        )
                        fs.append(f)
                    for j in range(NJ):
                        nc.tensor.matmul(
                            outp[j][:],
                            v_sb[:, t, g * 64 : (g + 1) * 64],
                            fs[j][:],
                            start=(t == 0), stop=(t == NSK - 1),
                        )
                for j in range(NJ):
                    ob = opool.tile([64, JW], F32, tag="ob")
                    nc.scalar.activation(ob[:], outp[j][:], ACT.Identity)
                    nc.sync.dma_start(
                        outT[h * 64 : (h + 1) * 64, j * JW : (j + 1) * JW], ob[:]
                    )

    nc.compile()
    return nc


_NC = None
LAST_EXEC_NS = None


def _get_nc():
    global _NC
    if _NC is None:
        _NC = _build()
    return _NC


def kernel(x, attention_mask, Wq, bq, Wk, bk, Wv, bv, **_ignored):
    x = np.asarray(x, dtype=np.float32)
    Wq = np.asarray(Wq, dtype=np.float32)
    Wk = np.asarray(Wk, dtype=np.float32)
    Wv = np.asarray(Wv, dtype=np.float32)
    bq = np.asarray(bq, dtype=np.float32)
    bk = np.asarray(bk, dtype=np.float32)
    bv = np.asarray(bv, dtype=np.float32)

    nc = _get_nc()
    in_maps = []
    for c in range(8):
        b, g = c // 4, c % 4
        qs = slice(g * DQ, (g + 1) * DQ)
        ks = slice(g * DKV, (g + 1) * DKV)
        in_maps.append(
            {
                "xT": np.ascontiguousarray(x[b].T),
                "wq": np.ascontiguousarray(Wq[:, qs]),
                "wk": np.ascontiguousarray(Wk[:, ks]),
                "wv": np.ascontiguousarray(Wv[:, ks]),
                "bq": np.ascontiguousarray(bq[qs]),
                "bk": np.ascontiguousarray(bk[ks]),
                "bv": np.ascontiguousarray(bv[ks]),
            }
        )
    import os
    trace = bool(int(os.environ.get("KERNEL_TRACE", "0")))
    res = bass_utils.run_bass_kernel_spmd(
        nc, in_maps, core_ids=list(range(8)), trace=trace,
        tmpdir=os.environ.get("KERNEL_TRACE_DIR") or None,
    )
    global LAST_EXEC_NS
    LAST_EXEC_NS = res.exec_time_ns
    out = np.empty((B, S, D), dtype=np.float32)
    for c in range(8):
        b, g = c // 4, c % 4
        out[b, :, g * DQ : (g + 1) * DQ] = res.results[c]["outT"].T
    return out
